# revision 15
# baseline (speedup 1.0000x reference)
"""Causal multi-head attention (B=2, S=2048, D=1024, H=16, hd=64) on 8 trn2 cores.

Sharding: core c handles batch b = c//4 and head group hg = c%4 (4 heads each).
Each core computes its Q/K/V shard (tensor-parallel columns of W_qkv), causal
attention for its 4 heads with scores held transposed ([s_k, s_q] so the PV
matmul needs no on-chip transposes), and a partial output projection over its
256 rows of W_proj. The host sums the 4 partials per batch and adds the exact
bias terms (softmax rows sum to 1, so attn@(V + 1 bv^T) = attn@V + bv^T; the
b_qkv V-slice and b_proj are applied on the host).

Matmul operands are bf16 (PSUM accumulation fp32); x is transposed/cast on the
host as part of sharding so the device needs no transposes at all.
"""

import numpy as np
import ml_dtypes
from contextlib import ExitStack

B, S, D, H = 2, 2048, 1024, 16
HD = 64
NCORES = 8
FPC = 256  # features per core (4 heads x 64)

_CACHE = {}


def _build():
    import concourse.bacc as bacc
    import concourse.tile as tile
    import concourse.mybir as mybir

    f32 = mybir.dt.float32
    bf16 = mybir.dt.bfloat16
    AF = mybir.ActivationFunctionType

    nc = bacc.Bacc("TRN2", target_bir_lowering=False, debug=False, num_devices=NCORES)

    xT = nc.dram_tensor("xT", [D, S], bf16, kind="ExternalInput").ap()
    wq = nc.dram_tensor("wq", [D, FPC], bf16, kind="ExternalInput").ap()
    wk = nc.dram_tensor("wk", [D, FPC], bf16, kind="ExternalInput").ap()
    wv = nc.dram_tensor("wv", [D, FPC], bf16, kind="ExternalInput").ap()
    wp = nc.dram_tensor("wp", [FPC, D], bf16, kind="ExternalInput").ap()
    bqk = nc.dram_tensor("bqk", [128, 4], f32, kind="ExternalInput").ap()
    maskT = nc.dram_tensor("maskT", [128, 128], bf16, kind="ExternalInput").ap()
    ident = nc.dram_tensor("ident", [128, 128], bf16, kind="ExternalInput").ap()
    out = nc.dram_tensor("out", [S, D], f32, kind="ExternalOutput").ap()

    with tile.TileContext(nc) as tc:
        with ExitStack() as ctx:
            _body(ctx, tc, mybir, out, xT, wq, wk, wv, wp, bqk, maskT, ident)

    nc.compile()
    return nc


def _body(ctx, tc, mybir, out, xT, wq, wk, wv, wp, bqk, maskT, ident):
    nc = tc.nc
    f32 = mybir.dt.float32
    bf16 = mybir.dt.bfloat16
    AF = mybir.ActivationFunctionType
    NK = D // 128   # 8 contraction tiles for qkv/proj-input dim
    NS = S // 128   # 16 sequence tiles

    sb = ctx.enter_context(tc.tile_pool(name="sb", bufs=1))

    xt_t = [sb.tile([128, S], bf16, name=f"xt{k}", tag=f"xt{k}") for k in range(NK)]
    wq_t = [sb.tile([128, FPC], bf16, name=f"wqt{k}", tag=f"wqt{k}") for k in range(NK)]
    wk_t = [sb.tile([128, FPC], bf16, name=f"wkt{k}", tag=f"wkt{k}") for k in range(NK)]
    wv_t = [sb.tile([128, FPC], bf16, name=f"wvt{k}", tag=f"wvt{k}") for k in range(NK)]
    wp_t = [sb.tile([128, D], bf16, name=f"wpt{k}", tag=f"wpt{k}") for k in range(2)]
    qt_t = [sb.tile([128, S], bf16, name=f"qtt{f}", tag=f"qtt{f}") for f in range(2)]
    kt_t = [sb.tile([128, S], bf16, name=f"ktt{f}", tag=f"ktt{f}") for f in range(2)]
    v_t = [sb.tile([128, 4 * 65], bf16, name=f"vt{s}", tag=f"vt{s}") for s in range(NS)]
    ot_t = [sb.tile([128, S], bf16, name=f"ott{f}", tag=f"ott{f}") for f in range(2)]
    bqk_t = sb.tile([128, 4], f32, name="bqkt", tag="bqkt")
    mask_t = sb.tile([128, 128], bf16, name="maskt", tag="maskt")
    ident_t = sb.tile([128, 128], bf16, name="identt", tag="identt")

    p_pool = ctx.enter_context(tc.tile_pool(name="pp", bufs=4))
    rc_pool = ctx.enter_context(tc.tile_pool(name="rcp", bufs=2))
    oo_pool = ctx.enter_context(tc.tile_pool(name="oop", bufs=3))

    # ---- input DMAs ----
    for k in range(NK):
        nc.sync.dma_start(xt_t[k][:], xT[k * 128:(k + 1) * 128, :])
        nc.sync.dma_start(wq_t[k][:], wq[k * 128:(k + 1) * 128, :])
        nc.sync.dma_start(wk_t[k][:], wk[k * 128:(k + 1) * 128, :])
        nc.sync.dma_start(wv_t[k][:], wv[k * 128:(k + 1) * 128, :])
    for k in range(2):
        nc.sync.dma_start(wp_t[k][:], wp[k * 128:(k + 1) * 128, :])
    nc.sync.dma_start(bqk_t[:], bqk[:])
    nc.sync.dma_start(mask_t[:], maskT[:])
    nc.sync.dma_start(ident_t[:], ident[:])

    # Unified PSUM pools for every phase (no phase barriers): "sc" slots are
    # 2 banks each x2, "pv" slots 2 banks each x2 -> 8 banks total.
    scp = ctx.enter_context(tc.tile_pool(name="ps_sc", bufs=2, space="PSUM"))
    pvp = ctx.enter_context(tc.tile_pool(name="ps_pv", bufs=2, space="PSUM"))

    def qkt_group(dst, w_t, bcol, f, c2):
        """One [128,1024] accumulation group of the Q^T/K^T projection."""
        ps = scp.tile([128, 1024], f32, name="sc", tag="sc", bufs=2)
        for k in range(NK):
            for sp in range(2):
                nc.tensor.matmul(
                    ps[:, sp * 512:(sp + 1) * 512],
                    w_t[k][:, f * 128:(f + 1) * 128],
                    xt_t[k][:, c2 * 1024 + sp * 512: c2 * 1024 + (sp + 1) * 512],
                    start=(k == 0), stop=(k == NK - 1),
                )
        nc.vector.tensor_scalar_add(
            dst[f][:, c2 * 1024:(c2 + 1) * 1024], ps[:],
            bqk_t[:, bcol + f: bcol + f + 1],
        )

    def v_group(s):
        psv = scp.tile([128, FPC], f32, name="sc", tag="sc", bufs=2)
        for k in range(NK):
            nc.tensor.matmul(
                psv[:],
                xt_t[k][:, s * 128:(s + 1) * 128],
                wv_t[k][:],
                start=(k == 0), stop=(k == NK - 1),
            )
        v3 = v_t[s].rearrange("p (h c) -> p h c", h=4)
        nc.scalar.activation(
            v3[:, :, 0:64], psv.rearrange("p (h c) -> p h c", h=4)[:], AF.Copy
        )
        nc.vector.memset(v3[:, :, 64:65], 1.0)

    class AttnUnit:
        """Causal attention for head h over queries [half*1024, +1024)."""

        def __init__(self, h, half):
            self.h, self.half = h, half
            self.hp, self.hh = h // 2, h % 2
            self.r0 = self.hh * 64
            self.q0 = half * 1024
            self.ki_n = NS // 2 * (half + 1)
            self.pv = pvp.tile([128, 1024], f32, name="pv", tag="pv", bufs=2)

        def emit_ki(self, ki):
            q0, r0, h = self.q0, self.r0, self.h
            qt, kt = qt_t[self.hp], kt_t[self.hp]
            qs = max(ki * 128, q0)   # first unmasked q for this k block
            a0 = qs - q0             # local col offset in the 1024 tile
            diag = ki * 128 >= q0    # diagonal block lives in this half
            spans = [(a0, 512), (512, 1024)] if a0 < 512 else [(a0, 1024)]
            sc = scp.tile([128, 1024], f32, name="sc", tag="sc", bufs=2)
            for (a, b) in spans:
                has_diag = diag and a <= a0 < b
                nc.tensor.matmul(
                    sc[:, a:b],
                    kt[r0:r0 + 64, ki * 128:(ki + 1) * 128],
                    qt[r0:r0 + 64, q0 + a:q0 + b],
                    start=True, stop=not has_diag,
                )
                if has_diag:  # causal mask: accumulate -30208 into masked area
                    nc.tensor.matmul(
                        sc[:, a0:a0 + 128], ident_t[:], mask_t[:],
                        start=False, stop=True,
                    )
            P = p_pool.tile([128, 1024], bf16, name="P", tag="P", bufs=4)
            nc.scalar.activation(P[:, a0:1024], sc[:, a0:1024], AF.Exp,
                                 scale=float(HD) ** -0.5)
            for (a, b) in spans:
                # last k-block contributing to this psum bank
                last_ki = min(self.ki_n - 1, (q0 + b - 1) // 128)
                nc.tensor.matmul(
                    self.pv[0:65, a:b],
                    v_t[ki][:, h * 65:h * 65 + 65],
                    P[:, a:b],
                    start=(ki == 0), stop=(ki == last_ki),
                )

        def finish(self):
            pv = self.pv
            dcp = rc_pool.tile([1, 1024], f32, name="dcp", tag="dcp", bufs=2)
            nc.vector.tensor_copy(dcp[:], pv[64:65, 0:1024])
            rcp = rc_pool.tile([1, 1024], f32, name="rcp", tag="rcp", bufs=2)
            nc.vector.reciprocal_approx_fast(rcp[:], dcp[:])
            rbc = rc_pool.tile([64, 1024], f32, name="rbc", tag="rbc", bufs=2)
            nc.gpsimd.partition_broadcast(rbc[:], rcp[:], channels=64)
            nc.vector.tensor_mul(
                ot_t[self.hp][self.r0:self.r0 + 64, self.q0:self.q0 + 1024],
                pv[0:64, :], rbc[:],
            )

    def attn_pair(ha, hb, half):
        """Two heads' units interleaved at ki granularity (two chains in
        flight hide the scores->exp->PV latency)."""
        ua, ub = AttnUnit(ha, half), AttnUnit(hb, half)
        for ki in range(ua.ki_n):
            ua.emit_ki(ki)
            ub.emit_ki(ki)
        ua.finish()
        ub.finish()

    def proj_group(s):
        pj = pvp.tile([128, 1024], f32, name="pv", tag="pv", bufs=2)
        for nh in range(2):
            for k2 in range(2):
                nc.tensor.matmul(
                    pj[:, nh * 512:(nh + 1) * 512],
                    ot_t[k2][:, s * 128:(s + 1) * 128],
                    wp_t[k2][:, nh * 512:(nh + 1) * 512],
                    start=(k2 == 0), stop=(k2 == 1),
                )
        oo = oo_pool.tile([128, D], f32, name="oo", tag="oo", bufs=3)
        nc.vector.tensor_copy(oo[:], pj[:])
        nc.sync.dma_start(out[s * 128:(s + 1) * 128, :], oo[:])

    # Program order = scheduler priority. Prelude computes the f0 tiles of
    # Q^T/K^T plus all of V (PE-dense, warms HAM); the f1 tiles are emitted
    # as PE filler between the first attention stages (which are ACT-paced);
    # proj of a finished q-half fills the last stage's gaps.
    qkt_group(qt_t, wq_t, 0, 0, 0)
    qkt_group(kt_t, wk_t, 2, 0, 0)
    qkt_group(qt_t, wq_t, 0, 0, 1)
    qkt_group(kt_t, wk_t, 2, 0, 1)
    for s in range(NS):
        v_group(s)

    attn_pair(0, 1, 0)
    qkt_group(qt_t, wq_t, 0, 1, 0)
    qkt_group(kt_t, wk_t, 2, 1, 0)
    attn_pair(2, 3, 0)
    qkt_group(qt_t, wq_t, 0, 1, 1)
    qkt_group(kt_t, wk_t, 2, 1, 1)
    attn_pair(0, 1, 1)
    for s in range(8):       # half-0 outT complete: fills the last stage
        proj_group(s)
    attn_pair(2, 3, 1)
    for s in range(8, NS):
        proj_group(s)


def _in_maps(x, W_qkv, b_qkv, W_proj):
    bf = ml_dtypes.bfloat16
    maps = []
    # additive causal mask for the transposed diag block: keep k<=q,
    # kill k>q (strict lower triangle) with a large negative constant
    mask = (np.tril(np.ones((128, 128), np.float32), -1) * -30208.0).astype(bf)
    ident = np.eye(128, dtype=np.float32).astype(bf)
    for core in range(NCORES):
        b, hg = core // 4, core % 4
        cs = slice(hg * FPC, (hg + 1) * FPC)
        bq = b_qkv[cs].astype(np.float32)
        bk = b_qkv[D + hg * FPC: D + (hg + 1) * FPC].astype(np.float32)
        maps.append({
            "xT": np.ascontiguousarray(x[b].T).astype(bf),
            "wq": np.ascontiguousarray(W_qkv[:, cs]).astype(bf),
            "wk": np.ascontiguousarray(W_qkv[:, D + hg * FPC: D + (hg + 1) * FPC]).astype(bf),
            "wv": np.ascontiguousarray(W_qkv[:, 2 * D + hg * FPC: 2 * D + (hg + 1) * FPC]).astype(bf),
            "wp": np.ascontiguousarray(W_proj[hg * FPC:(hg + 1) * FPC, :]).astype(bf),
            "bqk": np.ascontiguousarray(
                np.stack([bq[0:128], bq[128:256], bk[0:128], bk[128:256]], axis=1)),
            "maskT": mask,
            "ident": ident,
        })
    return maps


def get_nc():
    if "nc" not in _CACHE:
        _CACHE["nc"] = _build()
    return _CACHE["nc"]


def _postprocess(partials, b_qkv, W_proj, b_proj):
    out = np.zeros((B, S, D), np.float32)
    for core in range(NCORES):
        out[core // 4] += partials[core]
    bv = np.asarray(b_qkv, np.float32)[2 * D:3 * D]
    out += bv @ np.asarray(W_proj, np.float32) + np.asarray(b_proj, np.float32)
    return out


def kernel(x, W_qkv, b_qkv, W_proj, b_proj, _trace=False):
    from concourse.bass_utils import run_bass_kernel_spmd

    x = np.asarray(x, np.float32)
    W_qkv = np.asarray(W_qkv, np.float32)
    b_qkv = np.asarray(b_qkv, np.float32)
    W_proj = np.asarray(W_proj, np.float32)
    b_proj = np.asarray(b_proj, np.float32)

    nc = get_nc()
    maps = _in_maps(x, W_qkv, b_qkv, W_proj)
    res = run_bass_kernel_spmd(nc, maps, list(range(NCORES)), trace=_trace)
    _CACHE["last_result"] = res
    partials = [res.results[c]["out"] for c in range(NCORES)]
    return _postprocess(partials, b_qkv, W_proj, b_proj)
